# revision 3
# baseline (speedup 1.0000x reference)
"""Trainium2 Bass kernel for nn_Decoder (2-layer GAT + domain BatchNorm).

Distribution (8 NeuronCores, SPMD single program):
- Nodes are partitioned into contiguous blocks of NPC=2560 per core
  (20000 nodes -> core 7's tail is virtual padding). Each core owns the
  destination side of every edge whose dst falls in its range.
- Layer-1 feature table T1 [20480, 520] is computed replicated on every
  core (cheap small matmul); per-edge gathers then stay core-local.
- Per-edge work is done 128 edges at a time: indirect-DMA gather of
  source rows, selection-matrix matmul on the PE array performs the
  exp-weighted segment sum (edge softmax denominator rides along as a
  ones column in the table).
- Domain BatchNorm statistics are AllReduced (tiny); the layer-2 table
  is built from owned nodes only and AllGathered.

Table row layout (520 f32 = 2080 B, 32B-aligned):
  [0:256]   feat half 0 (head0 / feat2[0:256])
  [256]     1.0  (softmax denominator column)
  [257:513] feat half 1 (head1 / feat2[256:512]), col 513 = 1.0
  [514:516] el (per head; duplicated for layer 2)
  [516:518] er (per head; duplicated for layer 2)
  [518:520] pad
"""

from contextlib import ExitStack

import numpy as np

from concourse import bass, mybir
import concourse.tile as tile

from bass_compat import split_multi_waits, make_runner

F32 = mybir.dt.float32
I32 = mybir.dt.int32
AF = mybir.ActivationFunctionType
OP = mybir.AluOpType

P = 128
N_NODES = 20000
N_EDGES = 320000
H_DIM = 32
HID = 256
IN_DIM = 512
NDOM = 4
NCORES = 8
NPC = 2560                # padded nodes per core
NV = NCORES * NPC         # 20480 padded total
NBLK = NPC // P           # 20 dst blocks per core
R = 520                   # table row floats
NEG_SLOPE = 0.2
EPS = 1e-5

_cache = {}


def _build_program(Tb):
    """Build the SPMD Bass program. Tb: tile count per dst-block position
    (len NBLK, shared by all cores)."""
    T_total = sum(Tb)
    nc = bass.Bass()

    # ---------------- I/O ----------------
    xT_in = nc.dram_tensor("xT", [H_DIM, NV], F32, kind="ExternalInput")
    W1_in = nc.dram_tensor("W1", [H_DIM, 2 * HID], F32, kind="ExternalInput")
    al1_in = nc.dram_tensor("al1", [H_DIM, 2 * HID], F32, kind="ExternalInput")
    ar1_in = nc.dram_tensor("ar1", [H_DIM, 2 * HID], F32, kind="ExternalInput")
    W2a_in = nc.dram_tensor("W2a", [P, IN_DIM], F32, kind="ExternalInput")
    W2b_in = nc.dram_tensor("W2b", [P, IN_DIM], F32, kind="ExternalInput")
    al2_in = nc.dram_tensor("al2", [P, IN_DIM], F32, kind="ExternalInput")
    ar2_in = nc.dram_tensor("ar2", [P, IN_DIM], F32, kind="ExternalInput")
    gamma_in = nc.dram_tensor("gamma", [NDOM, HID], F32, kind="ExternalInput")
    beta_in = nc.dram_tensor("beta", [NDOM, HID], F32, kind="ExternalInput")
    b1m_in = nc.dram_tensor("b1m", [P, HID], F32, kind="ExternalInput")
    b2_in = nc.dram_tensor("b2", [P, IN_DIM], F32, kind="ExternalInput")
    ident_in = nc.dram_tensor("ident", [P, P], F32, kind="ExternalInput")
    iotaf_in = nc.dram_tensor("iotaf", [P, P], F32, kind="ExternalInput")
    iotac_in = nc.dram_tensor("iotac", [P, 1], F32, kind="ExternalInput")
    src_in = nc.dram_tensor("srcidx", [P, T_total], I32, kind="ExternalInput")
    dstl_in = nc.dram_tensor("dstl", [P, T_total], F32, kind="ExternalInput")
    own_in = nc.dram_tensor("ownids", [P, NBLK], I32, kind="ExternalInput")
    Yh_in = nc.dram_tensor("Yh", [P, NBLK * NDOM], F32, kind="ExternalInput")
    YhT_in = nc.dram_tensor("YhT", [NDOM, NPC], F32, kind="ExternalInput")

    out_relu = nc.dram_tensor("out_relu", [NPC, IN_DIM], F32, kind="ExternalOutput")
    out_sig = nc.dram_tensor("out_sig", [NPC, IN_DIM], F32, kind="ExternalOutput")

    # internal DRAM
    T1 = nc.dram_tensor("T1", [NV, R], F32)
    T2own = nc.dram_tensor("T2own", [NPC, R], F32)
    T2 = nc.dram_tensor("T2", [NV, R], F32, addr_space="Shared")
    bn_in_d = nc.dram_tensor("bn_in", [NDOM, 2 * HID + 1], F32)
    bn_out_d = nc.dram_tensor("bn_out", [NDOM, 2 * HID + 1], F32,
                              addr_space="Shared")

    rg = [list(range(NCORES))]

    with tile.TileContext(nc) as tc, ExitStack() as stack:
        cpool = stack.enter_context(tc.tile_pool(name="consts", bufs=1))

        def load_const(dram, shape, dtype=F32):
            t = cpool.tile(shape, dtype, tag=dram.name + "_c")
            nc.sync.dma_start(out=t[:], in_=dram[:])
            return t

        W1_sb = load_const(W1_in, [H_DIM, 2 * HID])
        al1_sb = load_const(al1_in, [H_DIM, 2 * HID])
        ar1_sb = load_const(ar1_in, [H_DIM, 2 * HID])
        W2a_sb = load_const(W2a_in, [P, IN_DIM])
        W2b_sb = load_const(W2b_in, [P, IN_DIM])
        al2_sb = load_const(al2_in, [P, IN_DIM])
        ar2_sb = load_const(ar2_in, [P, IN_DIM])
        gamma_sb = load_const(gamma_in, [NDOM, HID])
        beta_sb = load_const(beta_in, [NDOM, HID])
        b1m_sb = load_const(b1m_in, [P, HID])
        b2_sb = load_const(b2_in, [P, IN_DIM])
        ident_sb = load_const(ident_in, [P, P])
        iotaf_sb = load_const(iotaf_in, [P, P])
        iotac_sb = load_const(iotac_in, [P, 1])
        src_sb = load_const(src_in, [P, T_total], I32)
        dstl_sb = load_const(dstl_in, [P, T_total])
        own_sb = load_const(own_in, [P, NBLK], I32)
        Yh_sb = load_const(Yh_in, [P, NBLK * NDOM])
        YhT_sb = load_const(YhT_in, [NDOM, NPC])

        # persistent across phases
        o_all = cpool.tile([P, NBLK * HID], F32, tag="o_all")        # layer-1 output per block
        bn_acc = cpool.tile([NDOM, 2 * HID + 1], F32, tag="bn_acc")   # sums | count | sumsq
        nc.vector.memset(bn_acc[:], 0.0)

        # attention projection vectors: walwr[k, :] = [wl0 wl1 wr0 wr1]
        walwr = cpool.tile([H_DIM, 4], F32, tag="walwr")
        wlr2a = cpool.tile([P, 2], F32, tag="wlr2a")
        wlr2b = cpool.tile([P, 2], F32, tag="wlr2b")
        with tc.tile_pool(name="pre", bufs=2) as pre:
            t = pre.tile([H_DIM, 2 * HID], F32, tag="t")
            nc.vector.tensor_tensor(out=t[:], in0=W1_sb[:], in1=al1_sb[:], op=OP.mult)
            nc.vector.tensor_reduce(walwr[:, 0:1], t[:, 0:HID], mybir.AxisListType.X, OP.add)
            nc.vector.tensor_reduce(walwr[:, 1:2], t[:, HID:2 * HID], mybir.AxisListType.X, OP.add)
            t2 = pre.tile([H_DIM, 2 * HID], F32, tag="t")
            nc.vector.tensor_tensor(out=t2[:], in0=W1_sb[:], in1=ar1_sb[:], op=OP.mult)
            nc.vector.tensor_reduce(walwr[:, 2:3], t2[:, 0:HID], mybir.AxisListType.X, OP.add)
            nc.vector.tensor_reduce(walwr[:, 3:4], t2[:, HID:2 * HID], mybir.AxisListType.X, OP.add)
            u = pre.tile([P, IN_DIM], F32, tag="u")
            nc.vector.tensor_tensor(out=u[:], in0=W2a_sb[:], in1=al2_sb[:], op=OP.mult)
            nc.vector.tensor_reduce(wlr2a[:, 0:1], u[:], mybir.AxisListType.X, OP.add)
            u2 = pre.tile([P, IN_DIM], F32, tag="u")
            nc.vector.tensor_tensor(out=u2[:], in0=W2a_sb[:], in1=ar2_sb[:], op=OP.mult)
            nc.vector.tensor_reduce(wlr2a[:, 1:2], u2[:], mybir.AxisListType.X, OP.add)
            u3 = pre.tile([P, IN_DIM], F32, tag="u")
            nc.vector.tensor_tensor(out=u3[:], in0=W2b_sb[:], in1=al2_sb[:], op=OP.mult)
            nc.vector.tensor_reduce(wlr2b[:, 0:1], u3[:], mybir.AxisListType.X, OP.add)
            u4 = pre.tile([P, IN_DIM], F32, tag="u")
            nc.vector.tensor_tensor(out=u4[:], in0=W2b_sb[:], in1=ar2_sb[:], op=OP.mult)
            nc.vector.tensor_reduce(wlr2b[:, 1:2], u4[:], mybir.AxisListType.X, OP.add)

        # ---------------- Phase 1: build T1 (replicated) ----------------
        NSC = 8                       # superchunks of NPC columns
        with (
            tc.tile_pool(name="p1xt", bufs=2) as p1xt,
            tc.tile_pool(name="p1row", bufs=3) as p1row,
            tc.tile_pool(name="p1ps", bufs=2, space="PSUM") as p1ps,
            tc.tile_pool(name="p1ps2", bufs=2, space="PSUM") as p1ps2,
        ):
            for sc in range(NSC):
                xTs = p1xt.tile([H_DIM, NPC], F32)
                nc.sync.dma_start(out=xTs[:], in_=xT_in[:, sc * NPC:(sc + 1) * NPC])
                for j in range(NBLK):
                    lhs = xTs[:, j * P:(j + 1) * P]
                    feat_ps = p1ps.tile([P, 2 * HID], F32, space="PSUM")
                    nc.tensor.matmul(out=feat_ps[:], lhsT=lhs, rhs=W1_sb[:],
                                     start=True, stop=True)
                    elr_ps = p1ps2.tile([P, 4], F32, space="PSUM")
                    nc.tensor.matmul(out=elr_ps[:], lhsT=lhs, rhs=walwr[:],
                                     start=True, stop=True)
                    row = p1row.tile([P, R], F32)
                    nc.vector.tensor_copy(out=row[:, 0:HID], in_=feat_ps[:, 0:HID])
                    nc.vector.memset(row[:, HID:HID + 1], 1.0)
                    nc.vector.tensor_copy(out=row[:, HID + 1:2 * HID + 1],
                                          in_=feat_ps[:, HID:2 * HID])
                    nc.vector.memset(row[:, 2 * HID + 1:2 * HID + 2], 1.0)
                    nc.vector.tensor_copy(out=row[:, 514:518], in_=elr_ps[:, 0:4])
                    base = sc * NPC + j * P
                    nc.sync.dma_start(out=T1[base:base + P, :], in_=row[:])

        # ---------------- shared edge-phase body ----------------
        def edge_phase(table, two_heads, pools, epilogue):
            (gpool, mpool, spsum, papool, pbpool, smallp) = pools
            et = 0
            for b in range(NBLK):
                erb = gpool.tile([P, R], F32, tag="erb")
                nc.gpsimd.indirect_dma_start(
                    out=erb[:], out_offset=None, in_=table[:],
                    in_offset=bass.IndirectOffsetOnAxis(ap=own_sb[:, b:b + 1], axis=0),
                )
                pa = papool.tile([P, HID + 1], F32, space="PSUM")
                pb = pbpool.tile([P, HID + 1], F32, space="PSUM")
                for j in range(Tb[b]):
                    first = j == 0
                    last = j == Tb[b] - 1
                    g = gpool.tile([P, R], F32, tag="g")
                    nc.gpsimd.indirect_dma_start(
                        out=g[:], out_offset=None, in_=table[:],
                        in_offset=bass.IndirectOffsetOnAxis(
                            ap=src_sb[:, et:et + 1], axis=0),
                    )
                    dstf = dstl_sb[:, et:et + 1]
                    M = mpool.tile([P, P], F32, tag="M")
                    nc.vector.tensor_tensor(out=M[:], in0=dstf.to_broadcast([P, P]),
                                            in1=iotaf_sb[:], op=OP.is_equal)
                    trp = spsum.tile([P, P], F32, space="PSUM", tag="sp")
                    nc.tensor.transpose(out=trp[:], in_=dstf.to_broadcast([P, P]),
                                        identity=ident_sb[:])
                    dstT = mpool.tile([P, P], F32, tag="dstT")
                    nc.vector.tensor_copy(out=dstT[:], in_=trp[:])
                    MT = mpool.tile([P, P], F32, tag="MT")
                    nc.vector.tensor_tensor(out=MT[:], in0=dstT[:],
                                            in1=iotac_sb[:, 0:1].to_broadcast([P, P]),
                                            op=OP.is_equal)
                    er_ps = spsum.tile([P, 2], F32, space="PSUM", tag="sp")
                    nc.tensor.matmul(out=er_ps[:], lhsT=MT[:], rhs=erb[:, 516:518],
                                     start=True, stop=True)
                    e_sb = mpool.tile([P, 2], F32, tag="e")
                    nc.vector.tensor_tensor(out=e_sb[:], in0=g[:, 514:516],
                                            in1=er_ps[:], op=OP.add)
                    pe_sb = mpool.tile([P, 2], F32, tag="pe")
                    nc.scalar.activation(pe_sb[:], e_sb[:], AF.Prelu, alpha=NEG_SLOPE)
                    ee = mpool.tile([P, 2], F32, tag="ee")
                    nc.scalar.activation(ee[:], pe_sb[:], AF.Exp)
                    S0 = mpool.tile([P, P], F32, tag="S0")
                    nc.vector.tensor_scalar_mul(S0[:], M[:], ee[:, 0:1])
                    if two_heads:
                        S1 = mpool.tile([P, P], F32, tag="S1")
                        nc.vector.tensor_scalar_mul(S1[:], M[:], ee[:, 1:2])
                    else:
                        S1 = S0
                    nc.tensor.matmul(out=pa[:], lhsT=S0[:], rhs=g[:, 0:HID + 1],
                                     start=first, stop=last)
                    nc.tensor.matmul(out=pb[:], lhsT=S1[:],
                                     rhs=g[:, HID + 1:2 * HID + 2],
                                     start=first, stop=last)
                    et += 1
                epilogue(b, pa, pb, smallp)

        # ---------------- Phase 2: layer-1 edges ----------------
        with (
            tc.tile_pool(name="l1g", bufs=3) as gpool,
            tc.tile_pool(name="l1m", bufs=3) as mpool,
            tc.tile_pool(name="l1sp", bufs=3, space="PSUM") as spsum,
            tc.tile_pool(name="l1pa", bufs=2, space="PSUM") as papool,
            tc.tile_pool(name="l1pb", bufs=2, space="PSUM") as pbpool,
            tc.tile_pool(name="l1sm", bufs=3) as smallp,
        ):
            def epi1(b, pa, pb, smallp):
                s0c = smallp.tile([P, 1], F32, tag="s0c")
                nc.vector.tensor_scalar_max(s0c[:], pa[:, HID:HID + 1], 1e-30)
                r0 = smallp.tile([P, 1], F32, tag="r0")
                nc.vector.reciprocal(r0[:], s0c[:])
                r0h = smallp.tile([P, 1], F32, tag="r0h")
                nc.vector.tensor_scalar_mul(r0h[:], r0[:], 0.5)
                s1c = smallp.tile([P, 1], F32, tag="s1c")
                nc.vector.tensor_scalar_max(s1c[:], pb[:, HID:HID + 1], 1e-30)
                r1 = smallp.tile([P, 1], F32, tag="r1")
                nc.vector.reciprocal(r1[:], s1c[:])
                r1h = smallp.tile([P, 1], F32, tag="r1h")
                nc.vector.tensor_scalar_mul(r1h[:], r1[:], 0.5)
                t0 = smallp.tile([P, HID], F32, tag="t0")
                nc.vector.tensor_scalar_mul(t0[:], pa[:, 0:HID], r0h[:, 0:1])
                t1 = smallp.tile([P, HID], F32, tag="t1")
                nc.vector.tensor_scalar_mul(t1[:], pb[:, 0:HID], r1h[:, 0:1])
                t2_ = smallp.tile([P, HID], F32, tag="t2")
                nc.vector.tensor_tensor(out=t2_[:], in0=t0[:], in1=t1[:], op=OP.add)
                ob = o_all[:, b * HID:(b + 1) * HID]
                nc.vector.tensor_tensor(out=ob, in0=t2_[:], in1=b1m_sb[:], op=OP.add)
                # BN stats
                oext = smallp.tile([P, HID + 1], F32, tag="oext")
                nc.vector.tensor_copy(out=oext[:, 0:HID], in_=ob)
                nc.vector.memset(oext[:, HID:HID + 1], 1.0)
                osq = smallp.tile([P, HID], F32, tag="osq")
                nc.vector.tensor_tensor(out=osq[:], in0=ob, in1=ob, op=OP.mult)
                bnp = spsum.tile([NDOM, HID + 1], F32, space="PSUM", tag="sp")
                nc.tensor.matmul(out=bnp[:], lhsT=Yh_sb[:, b * NDOM:(b + 1) * NDOM],
                                 rhs=oext[:], start=True, stop=True)
                bnp2 = spsum.tile([NDOM, HID], F32, space="PSUM", tag="sp")
                nc.tensor.matmul(out=bnp2[:], lhsT=Yh_sb[:, b * NDOM:(b + 1) * NDOM],
                                 rhs=osq[:], start=True, stop=True)
                nc.vector.tensor_tensor(out=bn_acc[:, 0:HID + 1],
                                        in0=bn_acc[:, 0:HID + 1], in1=bnp[:], op=OP.add)
                nc.vector.tensor_tensor(out=bn_acc[:, HID + 1:2 * HID + 1],
                                        in0=bn_acc[:, HID + 1:2 * HID + 1],
                                        in1=bnp2[:], op=OP.add)

            edge_phase(T1, True, (gpool, mpool, spsum, papool, pbpool, smallp), epi1)

        # ---------------- Phase 2b: BN AllReduce + coefficients ----------------
        coefT = cpool.tile([NDOM, 2 * HID], F32, tag="coefT")
        with tc.tile_pool(name="bnp", bufs=1) as bnpool:
            nc.gpsimd.dma_start(out=bn_in_d[:], in_=bn_acc[:])
            nc.gpsimd.collective_compute(
                "AllReduce", OP.add, replica_groups=rg,
                ins=[bn_in_d[:]], outs=[bn_out_d[:]],
            )
            bnr = bnpool.tile([NDOM, 2 * HID + 1], F32)
            nc.gpsimd.dma_start(out=bnr[:], in_=bn_out_d[:])
            cnt = bnpool.tile([NDOM, 1], F32)
            nc.vector.tensor_scalar_max(cnt[:], bnr[:, HID:HID + 1], 1.0)
            rc = bnpool.tile([NDOM, 1], F32)
            nc.vector.reciprocal(rc[:], cnt[:])
            mu = bnpool.tile([NDOM, HID], F32)
            nc.vector.tensor_scalar_mul(mu[:], bnr[:, 0:HID], rc[:, 0:1])
            ex2 = bnpool.tile([NDOM, HID], F32)
            nc.vector.tensor_scalar_mul(ex2[:], bnr[:, HID + 1:2 * HID + 1], rc[:, 0:1])
            musq = bnpool.tile([NDOM, HID], F32)
            nc.vector.tensor_tensor(out=musq[:], in0=mu[:], in1=mu[:], op=OP.mult)
            var = bnpool.tile([NDOM, HID], F32)
            nc.vector.tensor_tensor(out=var[:], in0=ex2[:], in1=musq[:], op=OP.subtract)
            vepst = bnpool.tile([NDOM, HID], F32)
            nc.vector.tensor_scalar_add(vepst[:], var[:], EPS)
            sd = bnpool.tile([NDOM, HID], F32)
            nc.scalar.activation(sd[:], vepst[:], AF.Sqrt)
            rstd = bnpool.tile([NDOM, HID], F32)
            nc.vector.reciprocal(rstd[:], sd[:])
            nc.vector.tensor_tensor(out=coefT[:, 0:HID], in0=gamma_sb[:],
                                    in1=rstd[:], op=OP.mult)
            amu = bnpool.tile([NDOM, HID], F32)
            nc.vector.tensor_tensor(out=amu[:], in0=coefT[:, 0:HID], in1=mu[:],
                                    op=OP.mult)
            nc.vector.tensor_tensor(out=coefT[:, HID:2 * HID], in0=beta_sb[:],
                                    in1=amu[:], op=OP.subtract)

        # ---------------- Phase 3: normalize + feat2 (own nodes) ----------------
        with (
            tc.tile_pool(name="p3", bufs=3) as p3,
            tc.tile_pool(name="p3ab", bufs=2, space="PSUM") as p3ab,
            tc.tile_pool(name="p3f", bufs=2, space="PSUM") as p3f,
            tc.tile_pool(name="p3t", bufs=2, space="PSUM") as p3t,
            tc.tile_pool(name="p3e", bufs=2, space="PSUM") as p3e,
        ):
            for b in range(NBLK):
                ab_ps = p3ab.tile([P, 2 * HID], F32, space="PSUM")
                nc.tensor.matmul(out=ab_ps[:], lhsT=YhT_sb[:, b * P:(b + 1) * P],
                                 rhs=coefT[:], start=True, stop=True)
                ob = o_all[:, b * HID:(b + 1) * HID]
                t = p3.tile([P, HID], F32, tag="t")
                nc.vector.tensor_tensor(out=t[:], in0=ob, in1=ab_ps[:, 0:HID],
                                        op=OP.mult)
                on = p3.tile([P, HID], F32, tag="on")
                nc.vector.tensor_tensor(out=on[:], in0=t[:], in1=ab_ps[:, HID:2 * HID],
                                        op=OP.add)
                onl = p3.tile([P, HID], F32, tag="onl")
                nc.scalar.activation(onl[:], on[:], AF.Lrelu)  # slope 0.01
                onT = []
                for h in range(2):
                    trp = p3t.tile([P, P], F32, space="PSUM", tag="trp")
                    nc.tensor.transpose(out=trp[:], in_=onl[:, h * P:(h + 1) * P],
                                        identity=ident_sb[:])
                    ot = p3.tile([P, P], F32, tag=f"onT{h}")
                    nc.vector.tensor_copy(out=ot[:], in_=trp[:])
                    onT.append(ot)
                f2 = p3f.tile([P, IN_DIM], F32, space="PSUM")
                nc.tensor.matmul(out=f2[:], lhsT=onT[0][:], rhs=W2a_sb[:],
                                 start=True, stop=False)
                nc.tensor.matmul(out=f2[:], lhsT=onT[1][:], rhs=W2b_sb[:],
                                 start=False, stop=True)
                e2 = p3e.tile([P, 2], F32, space="PSUM")
                nc.tensor.matmul(out=e2[:], lhsT=onT[0][:], rhs=wlr2a[:],
                                 start=True, stop=False)
                nc.tensor.matmul(out=e2[:], lhsT=onT[1][:], rhs=wlr2b[:],
                                 start=False, stop=True)
                row = p3.tile([P, R], F32, tag="row")
                nc.vector.tensor_copy(out=row[:, 0:HID], in_=f2[:, 0:HID])
                nc.vector.memset(row[:, HID:HID + 1], 1.0)
                nc.vector.tensor_copy(out=row[:, HID + 1:2 * HID + 1],
                                      in_=f2[:, HID:2 * HID])
                nc.vector.memset(row[:, 2 * HID + 1:2 * HID + 2], 1.0)
                nc.vector.tensor_copy(out=row[:, 514:516],
                                      in_=e2[:, 0:1].to_broadcast([P, 2]))
                nc.vector.tensor_copy(out=row[:, 516:518],
                                      in_=e2[:, 1:2].to_broadcast([P, 2]))
                nc.sync.dma_start(out=T2own[b * P:(b + 1) * P, :], in_=row[:])

        # ---------------- Phase 3b: AllGather T2 ----------------
        nc.gpsimd.collective_compute(
            "AllGather", OP.bypass, replica_groups=rg,
            ins=[T2own[:]], outs=[T2[:]],
        )

        # ---------------- Phase 4: layer-2 edges ----------------
        with (
            tc.tile_pool(name="l2g", bufs=3) as gpool,
            tc.tile_pool(name="l2m", bufs=3) as mpool,
            tc.tile_pool(name="l2sp", bufs=3, space="PSUM") as spsum,
            tc.tile_pool(name="l2pa", bufs=2, space="PSUM") as papool,
            tc.tile_pool(name="l2pb", bufs=2, space="PSUM") as pbpool,
            tc.tile_pool(name="l2sm", bufs=3) as smallp,
        ):
            def epi2(b, pa, pb, smallp):
                sc_ = smallp.tile([P, 1], F32, tag="sc")
                nc.vector.tensor_scalar_max(sc_[:], pa[:, HID:HID + 1], 1e-30)
                rr = smallp.tile([P, 1], F32, tag="rr")
                nc.vector.reciprocal(rr[:], sc_[:])
                for half, ps in ((0, pa), (1, pb)):
                    oh = smallp.tile([P, HID], F32, tag=f"oh{half}")
                    nc.vector.tensor_scalar_mul(oh[:], ps[:, 0:HID], rr[:, 0:1])
                    obia = smallp.tile([P, HID], F32, tag=f"ob{half}")
                    nc.vector.tensor_tensor(
                        out=obia[:], in0=oh[:],
                        in1=b2_sb[:, half * HID:(half + 1) * HID], op=OP.add)
                    orl = smallp.tile([P, HID], F32, tag=f"or{half}")
                    nc.scalar.activation(orl[:], obia[:], AF.Relu)
                    osg = smallp.tile([P, HID], F32, tag=f"os{half}")
                    nc.scalar.activation(osg[:], orl[:], AF.Sigmoid)
                    nc.sync.dma_start(
                        out=out_relu[b * P:(b + 1) * P, half * HID:(half + 1) * HID],
                        in_=orl[:])
                    nc.sync.dma_start(
                        out=out_sig[b * P:(b + 1) * P, half * HID:(half + 1) * HID],
                        in_=osg[:])

            edge_phase(T2, False, (gpool, mpool, spsum, papool, pbpool, smallp), epi2)

    return nc


def _prepare(inputs):
    x = np.asarray(inputs["x"], np.float32)
    y = np.asarray(inputs["y"]).astype(np.int64)
    src = np.asarray(inputs["edge_src"]).astype(np.int64)
    dst = np.asarray(inputs["edge_dst"]).astype(np.int64)

    owner = dst // NPC
    order = np.argsort(dst, kind="stable")

    # per (core, block) edge lists
    tiles_needed = np.zeros((NCORES, NBLK), np.int64)
    per_cb = {}
    blk = (dst - owner * NPC) // P
    for c in range(NCORES):
        sel = order[owner[order] == c]
        bsel = blk[sel]
        for b in range(NBLK):
            e = sel[bsel == b]
            per_cb[(c, b)] = e
            tiles_needed[c, b] = (len(e) + P - 1) // P
    Tb = [max(1, int(tiles_needed[:, b].max())) for b in range(NBLK)]
    T_total = sum(Tb)

    src_pack = np.zeros((NCORES, P, T_total), np.int32)
    dstl_pack = np.full((NCORES, P, T_total), -1.0, np.float32)
    off = np.concatenate([[0], np.cumsum(Tb)])
    for c in range(NCORES):
        for b in range(NBLK):
            e = per_cb[(c, b)]
            n = len(e)
            col0 = off[b]
            if n:
                s = np.zeros(Tb[b] * P, np.int32)
                d = np.full(Tb[b] * P, -1.0, np.float32)
                s[:n] = src[e]
                d[:n] = (dst[e] - c * NPC - b * P).astype(np.float32)
                src_pack[c, :, col0:col0 + Tb[b]] = s.reshape(Tb[b], P).T
                dstl_pack[c, :, col0:col0 + Tb[b]] = d.reshape(Tb[b], P).T

    own_ids = np.zeros((NCORES, P, NBLK), np.int32)
    Yh = np.zeros((NCORES, P, NBLK * NDOM), np.float32)
    YhT = np.zeros((NCORES, NDOM, NPC), np.float32)
    for c in range(NCORES):
        ids = c * NPC + np.arange(NPC)
        own_ids[c] = ids.reshape(NBLK, P).T
        valid = ids < N_NODES
        yv = np.where(valid, y[np.minimum(ids, N_NODES - 1)], -1)
        onehot = (yv[:, None] == np.arange(NDOM)[None, :]).astype(np.float32)
        Yh[c] = onehot.reshape(NBLK, P, NDOM).transpose(1, 0, 2).reshape(P, NBLK * NDOM)
        YhT[c] = onehot.T

    xT = np.zeros((H_DIM, NV), np.float32)
    xT[:, :N_NODES] = x.T

    W1 = np.asarray(inputs["W1"], np.float32)
    attn_l1 = np.asarray(inputs["attn_l1"], np.float32)
    attn_r1 = np.asarray(inputs["attn_r1"], np.float32)
    bias1 = np.asarray(inputs["bias1"], np.float32)
    W2 = np.asarray(inputs["W2"], np.float32)
    attn_l2 = np.asarray(inputs["attn_l2"], np.float32)
    attn_r2 = np.asarray(inputs["attn_r2"], np.float32)
    bias2 = np.asarray(inputs["bias2"], np.float32)

    base = {
        "xT": xT,
        "W1": W1,
        "al1": np.tile(attn_l1.reshape(1, 2 * HID), (H_DIM, 1)),
        "ar1": np.tile(attn_r1.reshape(1, 2 * HID), (H_DIM, 1)),
        "W2a": W2[0:P, :],
        "W2b": W2[P:2 * P, :],
        "al2": np.tile(attn_l2.reshape(1, IN_DIM), (P, 1)),
        "ar2": np.tile(attn_r2.reshape(1, IN_DIM), (P, 1)),
        "gamma": np.asarray(inputs["gamma"], np.float32),
        "beta": np.asarray(inputs["beta"], np.float32),
        "b1m": np.tile((0.5 * (bias1[0:HID] + bias1[HID:2 * HID])).reshape(1, HID),
                       (P, 1)),
        "b2": np.tile(bias2.reshape(1, IN_DIM), (P, 1)),
        "ident": np.eye(P, dtype=np.float32),
        "iotaf": np.tile(np.arange(P, dtype=np.float32), (P, 1)),
        "iotac": np.arange(P, dtype=np.float32).reshape(P, 1),
    }
    in_maps = []
    for c in range(NCORES):
        m = dict(base)
        m["srcidx"] = src_pack[c]
        m["dstl"] = dstl_pack[c]
        m["ownids"] = own_ids[c]
        m["Yh"] = Yh[c]
        m["YhT"] = YhT[c]
        in_maps.append(m)
    return tuple(Tb), in_maps


def kernel(**inputs):
    Tb, in_maps = _prepare(inputs)
    if Tb not in _cache:
        nc = _build_program(list(Tb))
        split_multi_waits(nc)
        _cache[Tb] = make_runner(nc, NCORES)
    run = _cache[Tb]
    results, _t = run(in_maps, reps=1)

    o = np.empty((N_NODES, IN_DIM), np.float32)
    osig = np.empty((N_NODES, IN_DIM), np.float32)
    for c in range(NCORES):
        lo = c * NPC
        hi = min(lo + NPC, N_NODES)
        o[lo:hi] = results[c]["out_relu"][: hi - lo]
        osig[lo:hi] = results[c]["out_sig"][: hi - lo]
    return (o, osig)
